# revision 21
# baseline (speedup 1.0000x reference)
"""AngleLoss distributed Trainium2 kernel (v3).

mean(arccos(dot(o,t) / (|o||t|))) over 2,097,152 rows of 3-vectors,
data-parallel over 8 NeuronCores (no collective: each core returns
per-partition partial sums; host adds 16*128 floats per core).

Math per row (division- and sign-free, bf16 compute):
    dot = sum o*t ; oo = sum o^2 ; tt = sum t^2
    c    = dot * absrsqrt(oo*tt)              # cos(theta)
    nump = min(c-1, 0)                        # = -relu(1-c)
    r2   = absrsqrt(|1 - c^2|)
    g    = nump * r2                          # = -tan(theta/2)
    theta = -2*arctan(g), accumulated via accum_out.

Findings baked in (from HW traces):
  - bf16 inputs (host converts during shard): halves HBM traffic and all
    tensor_tensor ops hit the DVE 2x perf mode (0.52ns/el/lane measured);
    tensor_scalar hits 4x.
  - gpsimd tensor_tensor CONTENDS with DVE (~4x DVE slowdown while a pool
    op streams) -> pool does no compute here, it only issues input DMAs
    (its sequencer boots ~1.3us before sync's, so tile0 lands earlier).
  - teardown scales with semaphore count (~45-115ns per sem reset per
    engine, serialized) -> 6 semaphores total.
  - per front tile [P, 6, F] planar; products into a 9-plane buffer
    [m0 m1 m2 | ox2 oy2 oz2 | tx2 ty2 tz2]; two strided pair-adds
    (planes {0,3,6}+{1,4,7}, then +{2,5,8}) give [dot|oo|tt] planar.
  - single arctan at the end (one table switch, one accumulator drain);
    tapered fronts and chunks keep the drain chain short.
"""

import os as _os
import sys

import numpy as np

if "/opt/trn_rl_repo" not in sys.path:
    sys.path.insert(0, "/opt/trn_rl_repo")

N_CORES = 8
R_TOTAL = 256 * 8192  # 2097152 rows
PER_CORE = R_TOTAL // N_CORES  # 262144
P = 128
FREE = PER_CORE // P  # 2048


def _env_tuple(name, default):
    v = _os.environ.get(name)
    return tuple(int(x) for x in v.split(",")) if v else default


FRONT = _env_tuple("ANGLE_FRONT", (192, 512, 512, 512, 320))
CHUNK = _env_tuple("ANGLE_CHUNK", (896, 768, 384))
# square planes computed on VE (0..5); the remaining 6-SQ_ON_VE on Act
SQ_ON_VE = int(_os.environ.get("ANGLE_SQ_ON_VE", "1"))
# which engine issues input DMAs (gpsimd-issued DMA measured 7us slower)
DMA_ENG = _os.environ.get("ANGLE_DMA_ENG", "sync")
# skip the out-DMA completion wait: block teardown's dma_reset drains it
SKIP_DMO_WAIT = int(_os.environ.get("ANGLE_SKIP_DMO", "0"))
assert sum(FRONT) == FREE and sum(CHUNK) == FREE
assert 1 <= SQ_ON_VE <= 5 and len(CHUNK) >= 2

_BUILD_CACHE = {}


def _build_nc():
    key = (FRONT, CHUNK, SQ_ON_VE, DMA_ENG, SKIP_DMO_WAIT)
    if key in _BUILD_CACHE:
        return _BUILD_CACHE[key]

    from concourse import bacc, mybir

    AF = mybir.ActivationFunctionType
    OP = mybir.AluOpType
    f32 = mybir.dt.float32
    bf16 = mybir.dt.bfloat16

    T = len(FRONT)
    C = len(CHUNK)
    Fmax = max(FRONT)
    ofs = [0]
    for s in FRONT:
        ofs.append(ofs[-1] + s)
    cfs = [0]
    for s in CHUNK:
        cfs.append(cfs[-1] + s)
    need_b = []  # chunk j needs this many b-tiles
    for j in range(C):
        n = 0
        while ofs[n] < cfs[j + 1]:
            n += 1
        need_b.append(n)

    nc = bacc.Bacc(
        "TRN2", target_bir_lowering=False, debug=False, num_devices=N_CORES
    )
    x = nc.dram_tensor("x", [6 * P * FREE], bf16, kind="ExternalInput")
    out = nc.dram_tensor("out", [P, 16], f32, kind="ExternalOutput")
    xf = x.ap()

    def sb(name, shape, dtype):
        return nc.alloc_sbuf_tensor(name, list(shape), dtype).ap()

    inb = [sb(f"inb{b}", [P, 6 * Fmax], bf16) for b in range(3)]
    pl = [sb(f"pl{b}", [P, 9 * Fmax], bf16) for b in range(2)]
    ab = [sb(f"ab{b}", [P, 3 * Fmax], bf16) for b in range(2)]
    B = sb("B", [P, 3 * FREE], bf16)
    prodb = sb("prodb", [P, FREE], bf16)
    r1b = sb("r1b", [P, FREE], bf16)
    cb = sb("cb", [P, FREE], bf16)
    c2v = sb("c2v", [P, FREE], bf16)
    numpb = sb("numpb", [P, FREE], bf16)
    r2b = sb("r2b", [P, FREE], bf16)
    gall = sb("gall", [P, FREE], bf16)
    tscr = sb("tscr", [P, FREE], bf16)
    asum = sb("asum", [P, 16], f32)
    warm = sb("warm", [P, 1], bf16)
    bias0 = sb("bias0", [P, 1], f32)
    bias1 = sb("bias1", [P, 1], f32)

    S_dq = nc.alloc_semaphore("s_dq")  # dma completions, +16 each
    S_vf = nc.alloc_semaphore("s_vf")  # VE: memset(+1), then front_i (+1 ea)
    S_af = nc.alloc_semaphore("s_af")  # Act: sq_i (+1 each)
    S_vt = nc.alloc_semaphore("s_vt")  # VE progress: b/prod/c-group/g incs
    S_at = nc.alloc_semaphore("s_at")  # Act progress: r1/r2 incs
    S_fin = nc.alloc_semaphore("s_fin")
    S_dmo = nc.alloc_semaphore("s_dmo")

    B3 = B.rearrange("p (j f) -> p j f", j=3)  # planes dot|oo|tt

    def pl9(bidx, F):
        return pl[bidx][:, : 9 * F].rearrange("p (j f) -> p j f", j=9)

    def a3(bidx, F):
        return ab[bidx][:, : 3 * F].rearrange("p (j f) -> p j f", j=3)

    nsq_a = 6 - SQ_ON_VE

    # static positions of incs on the cross-engine progress sems
    vt_pos = {}  # name -> value after inc
    at_pos = {}
    vt_n = 0
    at_n = 0

    def vt_inc(name):
        nonlocal vt_n
        vt_n += 1
        vt_pos[name] = vt_n

    def at_inc(name):
        nonlocal at_n
        at_n += 1
        at_pos[name] = at_n

    # --- plan VE order (names) ---
    ve_order = []
    issued_b = 0
    np_, nc_, ng_ = 0, 0, 0

    def plan_tail():
        # prods ASAP; cgrps lag one prod (covers the r1 round trip on Act);
        # gs lag one cgrp (covers r2)
        nonlocal np_, nc_, ng_
        while np_ < C and need_b[np_] <= issued_b:
            ve_order.append(("prod", np_))
            np_ += 1
        while nc_ < np_ - 1:
            ve_order.append(("cgrp", nc_))
            nc_ += 1
        while ng_ < nc_ - 1:
            ve_order.append(("g", ng_))
            ng_ += 1

    for k in range(T):
        ve_order.append(("front", k))
        if k >= 1:
            ve_order.append(("b", k - 1))
            issued_b += 1
            plan_tail()
    ve_order.append(("b", T - 1))
    issued_b += 1
    plan_tail()
    while nc_ < C:
        ve_order.append(("cgrp", nc_))
        nc_ += 1
        while ng_ < nc_ - 1:
            ve_order.append(("g", ng_))
            ng_ += 1
    while ng_ < C:
        ve_order.append(("g", ng_))
        ng_ += 1
    # record vt positions in this order
    for st, idx in ve_order:
        if st == "b":
            vt_inc(f"b{idx}")
        elif st == "prod":
            vt_inc(f"prod{idx}")
        elif st == "cgrp":
            vt_inc(f"c2_{idx}")
        elif st == "g":
            vt_inc(f"g{idx}")

    # --- plan Act order ---
    act_order = []
    nr1, nr2 = 0, 0
    for i in range(T):
        act_order.append(("sq", i))
        ib = i - 1  # b-tiles guaranteed issued by VE before our wait
        while nr1 < C and need_b[nr1] <= ib:
            act_order.append(("r1", nr1))
            nr1 += 1
        while nr2 < nr1 - 1:
            act_order.append(("r2", nr2))
            nr2 += 1
    while nr1 < C:
        act_order.append(("r1", nr1))
        nr1 += 1
    while nr2 < C:
        act_order.append(("r2", nr2))
        nr2 += 1
    for st, idx in act_order:
        if st == "r1":
            at_inc(f"r1_{idx}")
        elif st == "r2":
            at_inc(f"r2_{idx}")

    with nc.Block(no_gpsimd_drain=True) as block:

        def emit_in_dmas(eng):
            for i in range(T):
                if i >= 3:
                    # inbuf reuse: tile i-3 consumed by BOTH fronts (separate
                    # sems: a merged counter can be satisfied lopsidedly and
                    # races the DMA over a buffer one engine still reads)
                    eng.wait_ge(S_vf, i - 1)
                    eng.wait_ge(S_af, i - 2)
                tile = xf[6 * P * ofs[i] : 6 * P * ofs[i + 1]].rearrange(
                    "(p f) -> p f", p=P
                )
                eng.dma_start(
                    out=inb[i % 3][:, : 6 * FRONT[i]], in_=tile
                ).then_inc(S_dq, 16)

        @block.sync
        def _(sync):
            if DMA_ENG == "sync":
                emit_in_dmas(sync)
            sync.wait_ge(S_fin, 1)
            sync.dma_start(out=out.ap()[:, :], in_=asum[:, :]).then_inc(
                S_dmo, 16
            )
            if not SKIP_DMO_WAIT:
                sync.wait_ge(S_dmo, 16)

        if DMA_ENG == "gpsimd":

            @block.gpsimd
            def _(gpsimd):
                emit_in_dmas(gpsimd)

        @block.vector
        def _(vector):
            vector.memset(bias0[:], 0.0)
            vector.memset(bias1[:], 1.0)
            vector.memset(asum[:, :], 0.0).then_inc(S_vf)

            def front(i):
                F = FRONT[i]
                vector.wait_ge(S_dq, 16 * (i + 1))
                vector.tensor_tensor(
                    pl[i % 2][:, : 3 * F],
                    inb[i % 3][:, : 3 * F],
                    inb[i % 3][:, 3 * F : 6 * F],
                    OP.mult,
                )
                vector.tensor_tensor(
                    pl[i % 2][:, 3 * F : (3 + SQ_ON_VE) * F],
                    inb[i % 3][:, : SQ_ON_VE * F],
                    inb[i % 3][:, : SQ_ON_VE * F],
                    OP.mult,
                ).then_inc(S_vf)

            def bstage(i):
                F = FRONT[i]
                p9 = pl9(i % 2, F)
                # a+b read Act's square planes of tile i
                vector.wait_ge(S_af, i + 1)
                vector.tensor_tensor(
                    a3(i % 2, F)[:], p9[:, 0:7:3, :], p9[:, 1:8:3, :], OP.add
                )
                vector.tensor_tensor(
                    B3[:, :, ofs[i] : ofs[i + 1]],
                    a3(i % 2, F)[:],
                    p9[:, 2:9:3, :],
                    OP.add,
                ).then_inc(S_vt)

            def prod(j):
                sl = slice(cfs[j], cfs[j + 1])
                vector.tensor_tensor(
                    prodb[:, sl], B3[:, 1, sl], B3[:, 2, sl], OP.mult
                ).then_inc(S_vt)

            def cgrp(j):
                sl = slice(cfs[j], cfs[j + 1])
                vector.wait_ge(S_at, at_pos[f"r1_{j}"])
                vector.tensor_tensor(
                    cb[:, sl], B3[:, 0, sl], r1b[:, sl], OP.mult
                )
                vector.tensor_scalar(
                    numpb[:, sl], cb[:, sl], 1.0, 0.0, OP.subtract, OP.min
                )
                vector.tensor_tensor(
                    c2v[:, sl], cb[:, sl], cb[:, sl], OP.mult
                ).then_inc(S_vt)

            def gstage(j):
                sl = slice(cfs[j], cfs[j + 1])
                vector.wait_ge(S_at, at_pos[f"r2_{j}"])
                vector.tensor_tensor(
                    gall[:, sl], numpb[:, sl], r2b[:, sl], OP.mult
                ).then_inc(S_vt)

            fns = {"front": front, "b": bstage, "prod": prod, "cgrp": cgrp,
                   "g": gstage}
            for st, idx in ve_order:
                fns[st](idx)

        @block.scalar
        def _(scalar):
            # first activation in program order pins the absrsqrt table set
            scalar.activation(
                warm[:], warm[:], AF.Abs_reciprocal_sqrt, bias=warm[:],
                scale=0.0,
            )
            scalar.wait_ge(S_vf, 1)

            def sq(i):
                F = FRONT[i]
                scalar.wait_ge(S_dq, 16 * (i + 1))
                if i >= 2:
                    # pl[i%2] square planes free once b of tile i-2 read them
                    scalar.wait_ge(S_vt, vt_pos[f"b{i - 2}"])
                scalar.activation(
                    pl[i % 2][:, (9 - nsq_a) * F : 9 * F],
                    inb[i % 3][:, (6 - nsq_a) * F : 6 * F],
                    AF.Square,
                    bias=bias0[:],
                ).then_inc(S_af)

            def r1(j):
                sl = slice(cfs[j], cfs[j + 1])
                scalar.wait_ge(S_vt, vt_pos[f"prod{j}"])
                scalar.activation(
                    r1b[:, sl], prodb[:, sl], AF.Abs_reciprocal_sqrt,
                    bias=bias0[:],
                ).then_inc(S_at)

            def r2(j):
                sl = slice(cfs[j], cfs[j + 1])
                scalar.wait_ge(S_vt, vt_pos[f"c2_{j}"])
                scalar.activation(
                    r2b[:, sl], c2v[:, sl], AF.Abs_reciprocal_sqrt,
                    bias=bias1[:], scale=-1.0,
                ).then_inc(S_at)

            fns = {"sq": sq, "r1": r1, "r2": r2}
            for st, idx in act_order:
                fns[st](idx)

            # dummy arctan: loads the sigmoid-set tables while VE finishes g
            scalar.activation(
                warm[:], warm[:], AF.Arctan, bias=bias0[:], scale=0.0
            )
            scalar.wait_ge(S_vt, vt_pos[f"g{C - 1}"])
            scalar.activation(
                tscr[:, :], gall[:, :], AF.Arctan, bias=bias0[:],
                accum_out=asum[:, 0:1],
            )
            # accumulator drains via a separate uop after ACTIVATE; two
            # trailing in-order ops carry the semaphore so the out-DMA
            # cannot read asum before the drain's SBUF write fully lands
            scalar.activation(
                warm[:], warm[:], AF.Copy, bias=0.0, scale=0.0
            )
            scalar.activation(
                warm[:], warm[:], AF.Copy, bias=0.0, scale=0.0
            ).then_inc(S_fin)

    nc.compile()
    _BUILD_CACHE[key] = nc
    return nc


def _shard_inputs(outputs, targets):
    import ml_dtypes

    bf = ml_dtypes.bfloat16
    o = np.asarray(outputs, dtype=np.float32).reshape(-1, 3)
    t = np.asarray(targets, dtype=np.float32).reshape(-1, 3)
    in_maps = []
    for cidx in range(N_CORES):
        lo, hi = cidx * PER_CORE, (cidx + 1) * PER_CORE
        planes = np.empty((6, P, FREE), dtype=bf)
        for k in range(3):
            planes[k] = o[lo:hi, k].astype(bf).reshape(P, FREE)
            planes[3 + k] = t[lo:hi, k].astype(bf).reshape(P, FREE)
        blocks = []
        off = 0
        for F in FRONT:
            blk = planes[:, :, off : off + F]  # [6, P, F]
            blocks.append(
                np.ascontiguousarray(blk.transpose(1, 0, 2)).reshape(-1)
            )
            off += F
        in_maps.append({"x": np.concatenate(blocks)})
    return in_maps


LAST_RESULT = None


def kernel(outputs, targets):
    global LAST_RESULT
    import os

    from concourse.bass_utils import run_bass_kernel_spmd

    nc = _build_nc()
    in_maps = _shard_inputs(outputs, targets)
    trace = bool(os.environ.get("ANGLE_KERNEL_TRACE"))
    res = run_bass_kernel_spmd(
        nc, in_maps, core_ids=list(range(N_CORES)), trace=trace
    )
    LAST_RESULT = res
    total = 0.0
    for rmap in res.results:
        total += np.asarray(rmap["out"], dtype=np.float64).sum()
    # device accumulates sum(arctan(-g)); theta = -2*arctan(g)
    mean = -2.0 * total / R_TOTAL
    return np.float32(mean)
